# revision 1
# baseline (speedup 1.0000x reference)
"""CenterLoss kernel for Trainium2 (8 NeuronCores, SPMD data-parallel over B).

Algorithm
---------
reference computes:
    counts[c] = #{i: y_i = c};  sums[c] = sum_{i: y_i = c} f_i
    means = sums / max(counts, 1);  present = counts > 0
    n_c = present ? 0.5*centers_c + 0.5*means_c : centers_c
    loss = 0.5 * mean_i ||f_i - n_{y_i}||^2

Expanding the loss (every class that appears in the batch is present):
    B * 2 * loss = S1 - 0.5*A - 0.75*X + 0.25*W
where
    S1 = sum_i ||f_i||^2
    A  = sum_c sums_c . centers_c
    X  = sum_{c present} ||sums_c||^2 / counts_c
    W  = sum_c counts_c * ||centers_c||^2

So the only heavy device work is the segment sums/counts over feats
(B=131072, D=256, C=1000) and S1.  Each core takes B/8 rows and computes:
  - partial segment sums+counts via one-hot matmuls on the PE
    (one-hot built on DVE from an iota table, feats converted fp32->fp16 on
    ACT; counts ride along as a 257th all-ones column of the rhs)
  - partial S1 via ACT Square activation with free-dim accumulation
The host sums the 8 partial [1024,257] tensors + 8 partial S1 vectors and
evaluates the tiny [C,D] closed form above (the gather/unshard step).
"""

import sys

sys.path.insert(0, "/opt/trn_rl_repo")

import numpy as np

# problem shape (hardcoded per the harness contract)
B, D, C = 131072, 256, 1000
N_CORES = 8
BS = B // N_CORES  # 16384 rows per core
P = 128
G = 4  # row-tiles per DMA group
TILES = BS // P  # 128
GROUPS = TILES // G  # 32
CPAD = 1024  # padded class count
CCHUNKS = CPAD // P  # 8
NFREE = D + 1  # 256 feat cols + 1 ones col for counts
FSTRIDE = 264  # fp16 sub-tile stride (4B aligned, 16B padded)
TAILG = 4  # trailing groups processed chunk-outer (store/compute overlap)

_CACHE: dict = {}


def _build_program():
    import concourse.bacc as bacc
    import concourse.bass as bass
    from concourse import mybir
    from concourse.tile import TileContext

    nc = bacc.Bacc("TRN2", target_bir_lowering=False)

    feats = nc.dram_tensor("feats", [BS, D], mybir.dt.float32, kind="ExternalInput")
    labels_in = nc.dram_tensor(
        "labels", [P, TILES], mybir.dt.float16, kind="ExternalInput"
    )
    # [128 x (8*257 sums+counts | 1 s1)]; stored per chunk so early stores
    # overlap the tail matmuls
    out_sums = nc.dram_tensor(
        "out_sums", [P, CCHUNKS * NFREE + 1], mybir.dt.float32, kind="ExternalOutput"
    )

    feats_ap = feats[:]

    with TileContext(nc) as tc:
        with (
            tc.tile_pool(name="const", bufs=1) as const,
            tc.tile_pool(name="fin", bufs=4) as fin,
            tc.tile_pool(name="sq", bufs=2) as sqp,
            tc.tile_pool(name="f16p", bufs=TAILG + 2) as f16p,
            tc.tile_pool(name="ohp", bufs=4 * TAILG + 6) as ohp,
            tc.tile_pool(name="accp", bufs=1) as accp,
            tc.tile_pool(name="psp", bufs=1, space="PSUM") as psp,
        ):
            # labels DMA (fp16, converted to fp32 on DVE: tensor_scalar
            # is_equal needs an fp32 scalar operand); iota built on the
            # otherwise-idle GPSIMD engine, converted int32 -> fp16 on DVE
            labels16_t = const.tile([P, TILES], mybir.dt.float16, tag="labels16_t")
            nc.sync.dma_start(out=labels16_t[:], in_=labels_in[:])
            labels_t = const.tile([P, TILES], mybir.dt.float32, tag="labels_t")
            nc.vector.tensor_copy(out=labels_t[:], in_=labels16_t[:])
            iota_i = const.tile([P, CPAD], mybir.dt.int32, tag="iota_i")
            nc.gpsimd.iota(iota_i[:], pattern=[[1, CPAD]], channel_multiplier=0)
            iota_f = const.tile([P, CPAD], mybir.dt.float16, tag="iota_f")
            nc.vector.tensor_copy(out=iota_f[:], in_=iota_i[:])
            iota_t = iota_f[:]

            tail_ohs, tail_f16gs = [], []
            # persistent accumulators
            # one column per (group, extra-half): 32 + 3 split extras
            s1cols = accp.tile([P, GROUPS + 3], mybir.dt.float32, tag="s1cols")
            s1_extra_col = [GROUPS]  # next free extra column
            psums = [
                psp.tile(
                    [P, NFREE], mybir.dt.float32, tag=f"psum{k}", name=f"psum{k}"
                )
                for k in range(CCHUNKS)
            ]
            # HAM warm-up: the PE runs at the cold 1.2 GHz clock until ~3.4us
            # of sustained activity. The head leaves PE idle until ~4.6us, so
            # the first ~19 real matmuls would run at half clock. Issue dummy
            # matmuls (zeroed operands, results discarded by the real
            # start=True PSUM clear) from ~0.5us so the real stream is warm.
            warm = const.tile([P, NFREE], mybir.dt.float16, tag="warm")
            nc.vector.memset(warm[:1, :1], 0.0)  # touch so Tile allocates it
            for w in range(12):
                nc.tensor.matmul(
                    out=psums[0][:],
                    lhsT=warm[:, 0:P],
                    rhs=warm[:],
                    start=True,
                    stop=True,
                )

            for t in range(GROUPS):
                # load a [P, G, D] group of feats rows (rows t*512 .. t*512+511).
                # Groups 0/1 are split into smaller loads/conversions so the
                # first matmul starts as soon as the first 128 rows land.
                f16g = f16p.tile([P, G, FSTRIDE], mybir.dt.float16, tag="f16g")
                if t == 0:
                    halves = ((0, 1), (1, 1), (2, 2))
                elif t == 1:
                    halves = ((0, 2), (2, 2))
                else:
                    halves = ((0, G),)
                for h, (off, gh) in enumerate(halves):
                    fg = fin.tile(
                        [P, gh, D], mybir.dt.float32, tag="fg", name="fg"
                    )
                    # very first load rides the ACT HWDGE ring so its
                    # descriptor-gen overlaps the labels DMA's on the SP ring
                    dma_eng = nc.scalar if t == 0 else nc.sync
                    dma_eng.dma_start(
                        out=fg[:],
                        in_=bass.AP(
                            tensor=feats_ap.tensor,
                            offset=(t * G + off) * P * D,
                            ap=[[D, P], [P * D, gh], [1, D]],
                        ),
                    )
                    # fp32 -> fp16 conversion (ACT)
                    nc.scalar.copy(
                        out=f16g[:, off : off + gh, 0:D], in_=fg[:]
                    )
                    # S1 partial: sum over free dim of feats^2 (ACT square+accum)
                    sqt = sqp.tile([P, gh, D], mybir.dt.float32, tag="sqt", name="sqt")
                    if h == 0:
                        col = t
                    else:
                        col = s1_extra_col[0]
                        s1_extra_col[0] += 1
                    nc.scalar.activation(
                        out=sqt[:],
                        in_=fg[:],
                        func=mybir.ActivationFunctionType.Square,
                        accum_out=s1cols[:, col : col + 1],
                    )
                # ones column for counts (DVE)
                nc.vector.memset(f16g[:, :, D : D + 1], 1.0)

                ohs = []
                for s in range(G):
                    j = t * G + s
                    oh = ohp.tile([P, CPAD], mybir.dt.float16, tag="oh")
                    nc.vector.tensor_scalar(
                        oh[:],
                        iota_t,
                        labels_t[:, j : j + 1],
                        None,
                        mybir.AluOpType.is_equal,
                    )
                    ohs.append(oh)
                if t < GROUPS - TAILG:
                    for s in range(G):
                        rhs = f16g[:, s, 0:NFREE]
                        for k in range(CCHUNKS):
                            nc.tensor.matmul(
                                out=psums[k][:],
                                lhsT=ohs[s][:, k * P : (k + 1) * P],
                                rhs=rhs,
                                start=(t == 0 and s == 0),
                                stop=False,
                            )
                else:
                    tail_ohs.append(ohs)
                    tail_f16gs.append(f16g)
            # last TAILG groups: chunk-outer order so chunk k's accumulation
            # closes early and its evacuation/store overlaps the remaining
            # chunks' matmuls
            for k in range(CCHUNKS):
                for g, (ohs_g, f16g_g) in enumerate(zip(tail_ohs, tail_f16gs)):
                    for s in range(G):
                        nc.tensor.matmul(
                            out=psums[k][:],
                            lhsT=ohs_g[s][:, k * P : (k + 1) * P],
                            rhs=f16g_g[:, s, 0:NFREE],
                            start=False,
                            stop=(g == TAILG - 1 and s == G - 1),
                        )

            # write back partials (PSUM -> SBUF -> DRAM; DMA can't read PSUM)
            ev = accp.tile([P, CCHUNKS * NFREE + 1], mybir.dt.float32, tag="ev")
            nc.vector.tensor_reduce(
                out=ev[:, CCHUNKS * NFREE : CCHUNKS * NFREE + 1],
                in_=s1cols[:],
                axis=mybir.AxisListType.X,
                op=mybir.AluOpType.add,
            )
            for k in range(CCHUNKS):
                dst = ev[:, k * NFREE : (k + 1) * NFREE]
                if k % 2 == 0:
                    nc.vector.tensor_copy(out=dst, in_=psums[k][:])
                else:
                    nc.scalar.copy(out=dst, in_=psums[k][:])
            # per-chunk stores: chunks close ~1.7 us apart (chunk-outer tail),
            # so early stores hide under compute and the last piece is small
            for k in range(CCHUNKS):
                lo = k * NFREE
                hi = (k + 1) * NFREE + (1 if k == CCHUNKS - 1 else 0)
                nc.sync.dma_start(out=out_sums[:, lo:hi], in_=ev[:, lo:hi])

    nc.compile()
    return nc


def _get_program():
    if "nc" not in _CACHE:
        _CACHE["nc"] = _build_program()
    return _CACHE["nc"]


def _run_device(feats_np: np.ndarray, labels_np: np.ndarray, trace: bool = False):
    """Shard over cores, run the SPMD bass kernel, return per-core results."""
    from concourse.bass_utils import run_bass_kernel_spmd

    nc = _get_program()
    in_maps = []
    for c in range(N_CORES):
        fshard = np.ascontiguousarray(feats_np[c * BS : (c + 1) * BS])
        lshard = labels_np[c * BS : (c + 1) * BS]
        # [P, TILES]; fp16 is exact for labels < 2048
        ltile = np.ascontiguousarray(lshard.reshape(TILES, P).T.astype(np.float16))
        in_maps.append({"feats": fshard, "labels": ltile})
    kw = {}
    if trace:
        kw = {"trace": True}
    try:
        return run_bass_kernel_spmd(nc, in_maps, core_ids=list(range(N_CORES)), **kw)
    except Exception:
        # transient axon/terminal faults have been observed; retry once
        import time

        time.sleep(2.0)
        return run_bass_kernel_spmd(nc, in_maps, core_ids=list(range(N_CORES)), **kw)


def kernel(feats, centers, labels, _trace: bool = False, _return_res: bool = False):
    feats = np.asarray(feats, dtype=np.float32)
    centers = np.asarray(centers, dtype=np.float32)
    labels_i = np.asarray(labels).astype(np.int64)

    res = _run_device(feats, labels_i, trace=_trace)

    # host combine (the gather/unshard step): tiny [C, D] math
    sums_all = np.zeros((CPAD, NFREE), dtype=np.float64)
    S1 = 0.0
    for c in range(N_CORES):
        raw = res.results[c]["out_sums"]
        part = (
            raw[:, : CCHUNKS * NFREE]
            .reshape(P, CCHUNKS, NFREE)
            .transpose(1, 0, 2)
            .reshape(CPAD, NFREE)
        )
        sums_all += part.astype(np.float64)
        S1 += float(raw[:, CCHUNKS * NFREE].sum())
    sums = sums_all[:C, :D]
    counts = sums_all[:C, D]

    c64 = centers.astype(np.float64)
    A = float((sums * c64).sum())
    present = counts > 0
    X = float((np.square(sums).sum(axis=1)[present] / counts[present]).sum())
    W = float((counts * np.square(c64).sum(axis=1)).sum())
    loss = 0.5 / B * (S1 - 0.5 * A - 0.75 * X + 0.25 * W)
    out = np.float32(loss)
    if _return_res:
        return out, res
    return out



# revision 2
# speedup vs baseline: 2.0031x; 2.0031x over previous
"""CenterLoss kernel for Trainium2 (8 NeuronCores, SPMD data-parallel over B).

Algorithm
---------
reference computes:
    counts[c] = #{i: y_i = c};  sums[c] = sum_{i: y_i = c} f_i
    means = sums / max(counts, 1);  present = counts > 0
    n_c = present ? 0.5*centers_c + 0.5*means_c : centers_c
    loss = 0.5 * mean_i ||f_i - n_{y_i}||^2

Expanding the loss (every class that appears in the batch is present):
    B * 2 * loss = S1 - 0.5*A - 0.75*X + 0.25*W
where
    S1 = sum_i ||f_i||^2
    A  = sum_c sums_c . centers_c
    X  = sum_{c present} ||sums_c||^2 / counts_c
    W  = sum_c counts_c * ||centers_c||^2

Device work per core (B/8 = 16384 rows): segment sums/counts over the
1000 classes plus S1, all via fp8e4 DoubleRow matmuls on the PE
(0.5 cycles/row, 256-deep contraction = 4x fp16 FLOP density):
  - feats are DMAed fp32 (46.6us at the 360GB/s model cap - the memory
    roofline) and converted fp32->fp8e4 on ACT.
  - the one-hot is built on DVE as (iota==label)*0.5 in fp16 (4x DVE
    mode, 327ns/row-tile); the fp16 byte pattern 0x3800 puts fp8e4-1.0
    (0x38) at odd byte offsets, so an fp8 bitcast view with stride 2 IS
    the fp8 one-hot.  DoubleRow requires contiguous weights, so feats
    column-chunks are the stationary operand and the strided one-hot is
    the moving operand: out[feat_col, class] accumulates transposed.
  - counts ride on an all-ones fp8 weights tile; S1 comes from the
    diagonal of the two 128x128 Gram self-product blocks, accumulated in
    a shared PSUM bank and extracted on the host.
PSUM: 7 banks (4 sums + 2 counts + 1 Gram), each zero-initialized by an
fp8 DoubleRow matmul on zeroed operands (start=True), then accumulated
into with start=False - correct under both region-zeroing and
per-address PSUM-start semantics.
The host sums the 8 per-core partials and evaluates the tiny closed form
above (the gather/unshard step).
"""

import sys

sys.path.insert(0, "/opt/trn_rl_repo")

import numpy as np

# problem shape (hardcoded per the harness contract)
B, D, C = 131072, 256, 1000
N_CORES = 8
BS = B // N_CORES  # 16384 rows per core
P = 128
TILES = BS // P  # 128 row-tiles
NDT = TILES // 2  # 64 double-tiles (256 rows each)
CPAD = 1024  # padded class count
HG = 32  # half-groups (2 double-tiles each) per core
FS = 272  # fp8 sub-tile stride (16B aligned for DoubleRow weights)
TAILD = 4  # trailing double-tiles processed bank-outer (store overlap)
EVC = 2304  # 4*512 sums + 2*128 Gram diag blocks

_CACHE: dict = {}


def _build_program():
    import concourse.bacc as bacc
    import concourse.bass as bass
    from concourse import mybir
    from concourse.tile import TileContext

    DR = mybir.MatmulPerfMode.DoubleRow
    nc = bacc.Bacc("TRN2", target_bir_lowering=False)

    feats = nc.dram_tensor("feats", [BS, D], mybir.dt.float32, kind="ExternalInput")
    labels_in = nc.dram_tensor(
        "labels", [P, TILES], mybir.dt.float32, kind="ExternalInput"
    )
    out_main = nc.dram_tensor("out_main", [P, EVC], mybir.dt.float16, kind="ExternalOutput")
    out_cnt = nc.dram_tensor("out_cnt", [1, CPAD], mybir.dt.float32, kind="ExternalOutput")

    feats_ap = feats[:]

    with TileContext(nc) as tc:
        with (
            tc.tile_pool(name="const", bufs=1) as const,
            tc.tile_pool(name="fin", bufs=4) as fin,
            tc.tile_pool(name="f8p", bufs=5) as f8p,
            tc.tile_pool(name="ohp", bufs=4 + 2 * TAILD) as ohp,
            tc.tile_pool(name="evp", bufs=1) as evp,
            tc.tile_pool(name="psp", bufs=1, space="PSUM") as psp,
        ):
            # labels ride the ACT HWDGE ring so the SP ring starts on feats
            # immediately; fp32 labels need no on-chip conversion
            labels_t = const.tile([P, TILES], mybir.dt.float32, tag="labels_t")
            nc.scalar.dma_start(out=labels_t[:], in_=labels_in[:])
            # iota built on the otherwise-idle GPSIMD engine, converted
            # int32 -> fp16 on ACT
            iota_i = const.tile([P, CPAD], mybir.dt.int32, tag="iota_i")
            nc.gpsimd.iota(iota_i[:], pattern=[[1, CPAD]], channel_multiplier=0)
            iota_f = const.tile([P, CPAD], mybir.dt.float16, tag="iota_f")
            nc.scalar.copy(out=iota_f[:], in_=iota_i[:])
            # fp8 constants: zeros for PSUM bank-init, ones for counts
            warm8 = const.tile([P, 2, 512], mybir.dt.float8e4, tag="warm8")
            nc.vector.memset(warm8[:], 0.0)
            ones8 = const.tile([P, 2, P], mybir.dt.float8e4, tag="ones8")
            nc.vector.memset(ones8[:], 1.0)

            # PSUM banks: sums[chunk][half] transposed ([feat_col, class]),
            # counts (replicated over partitions), Gram diag blocks
            ps_sums = [
                psp.tile([P, 512], mybir.dt.float32, tag=f"s{i}", name=f"s{i}")
                for i in range(4)  # order: A0, A1, B0, B1
            ]
            ps_cnt = [
                psp.tile([P, 512], mybir.dt.float32, tag=f"c{i}", name=f"c{i}")
                for i in range(2)
            ]
            ps_g = psp.tile([P, 512], mybir.dt.float32, tag="g", name="g")
            banks = ps_sums + ps_cnt + [ps_g]
            # bank-init doubles as PE p-state warmup: zeroes every bank with
            # the same fp8 DoubleRow mode the real stream uses
            for b in banks:
                nc.tensor.matmul(
                    out=b[:], lhsT=warm8[:, :, 0:P], rhs=warm8[:, :, 0:512],
                    perf_mode=DR, start=True, stop=False,
                )

            def issue_dt(oh8, f8, g, stops=(False,) * 7):
                """8 matmuls for one double-tile; f8 slice dim1 = [2g, 2g+2)."""
                fsl = f8[:, 2 * g : 2 * g + 2, :]
                for half in range(2):
                    rhs = oh8[:, :, half * 512 : (half + 1) * 512]
                    for ci in range(2):
                        nc.tensor.matmul(
                            out=ps_sums[2 * ci + half][:],
                            lhsT=fsl[:, :, ci * P : (ci + 1) * P],
                            rhs=rhs, perf_mode=DR,
                            start=False, stop=stops[2 * ci + half],
                        )
                    nc.tensor.matmul(
                        out=ps_cnt[half][:], lhsT=ones8[:], rhs=rhs,
                        perf_mode=DR, start=False, stop=stops[4 + half],
                    )
                for ci in range(2):
                    nc.tensor.matmul(
                        out=ps_g[:, ci * P : (ci + 1) * P],
                        lhsT=fsl[:, :, ci * P : (ci + 1) * P],
                        rhs=fsl[:, :, ci * P : (ci + 1) * P],
                        perf_mode=DR, start=False,
                        stop=stops[6] and ci == 1,
                    )

            def build_oh(dt):
                oh = ohp.tile([P, 2, CPAD], mybir.dt.float16, tag="oh")
                for kt in range(2):
                    j = 2 * dt + kt
                    # (iota==label)*0.5 in fp16: bytes [0x00, 0x38] put
                    # fp8e4-1.0 at odd offsets -> stride-2 fp8 view below
                    nc.vector.tensor_scalar(
                        oh[:, kt, :], iota_f[:], labels_t[:, j : j + 1], 0.5,
                        mybir.AluOpType.is_equal, mybir.AluOpType.mult,
                    )
                return oh[:].bitcast(mybir.dt.float8e4)[:, :, 1::2]

            tail_oh8s, tail_f8s = [], []
            for hg in range(HG):
                # load a [P, 4, D] half-group (rows hg*512 .. hg*512+511);
                # the first is split so the pipeline fills faster
                f8 = f8p.tile([P, 4, FS], mybir.dt.float8e4, tag="f8")
                parts = ((0, 2), (2, 2)) if hg == 0 else ((0, 4),)
                for off, gh in parts:
                    fg = fin.tile([P, gh, D], mybir.dt.float32, tag="fg", name="fg")
                    nc.sync.dma_start(
                        out=fg[:],
                        in_=bass.AP(
                            tensor=feats_ap.tensor,
                            offset=(hg * 4 + off) * P * D,
                            ap=[[D, P], [P * D, gh], [1, D]],
                        ),
                    )
                    # fp32 -> fp8e4 conversion (ACT)
                    nc.scalar.copy(out=f8[:, off : off + gh, 0:D], in_=fg[:])
                for g in range(2):
                    dt = 2 * hg + g
                    oh8 = build_oh(dt)
                    if dt < NDT - TAILD:
                        issue_dt(oh8, f8, g)
                    else:
                        tail_oh8s.append(oh8)
                        tail_f8s.append((f8, g))

            # tail: bank-outer order so each bank closes early and its
            # evacuation/store overlaps the remaining banks' matmuls
            ev = evp.tile([P, EVC], mybir.dt.float16, tag="ev")
            cv = evp.tile([1, CPAD], mybir.dt.float32, tag="cv")
            nstores = []
            for bi in range(7):
                for ti in range(TAILD):
                    last = ti == TAILD - 1
                    oh8 = tail_oh8s[ti]
                    f8, g = tail_f8s[ti]
                    fsl = f8[:, 2 * g : 2 * g + 2, :]
                    if bi < 4:
                        half, ci = bi & 1, bi >> 1
                        nc.tensor.matmul(
                            out=ps_sums[2 * ci + half][:],
                            lhsT=fsl[:, :, ci * P : (ci + 1) * P],
                            rhs=oh8[:, :, half * 512 : (half + 1) * 512],
                            perf_mode=DR, start=False, stop=last,
                        )
                    elif bi < 6:
                        half = bi - 4
                        nc.tensor.matmul(
                            out=ps_cnt[half][:], lhsT=ones8[:],
                            rhs=oh8[:, :, half * 512 : (half + 1) * 512],
                            perf_mode=DR, start=False, stop=last,
                        )
                    else:
                        for ci in range(2):
                            nc.tensor.matmul(
                                out=ps_g[:, ci * P : (ci + 1) * P],
                                lhsT=fsl[:, :, ci * P : (ci + 1) * P],
                                rhs=fsl[:, :, ci * P : (ci + 1) * P],
                                perf_mode=DR, start=False,
                                stop=last and ci == 1,
                            )
                # evacuate + store the closed bank (ACT copy, SP DMA)
                if bi < 4:
                    half, ci = bi & 1, bi >> 1
                    lo = (2 * ci + half) * 512
                    nc.scalar.copy(out=ev[:, lo : lo + 512], in_=ps_sums[2 * ci + half][:])
                    nc.sync.dma_start(out=out_main[:, lo : lo + 512], in_=ev[:, lo : lo + 512])
                elif bi < 6:
                    half = bi - 4
                    nc.scalar.copy(
                        out=cv[0:1, half * 512 : (half + 1) * 512],
                        in_=ps_cnt[half][0:1, :],
                    )
                    if bi == 5:
                        nc.sync.dma_start(out=out_cnt[:], in_=cv[:])
                else:
                    nc.scalar.copy(out=ev[:, 2048:2304], in_=ps_g[:, 0:256])
                    nc.sync.dma_start(out=out_main[:, 2048:2304], in_=ev[:, 2048:2304])

    nc.compile()
    return nc


def _get_program():
    if "nc" not in _CACHE:
        _CACHE["nc"] = _build_program()
    return _CACHE["nc"]


def _run_device(feats_np: np.ndarray, labels_np: np.ndarray, trace: bool = False):
    """Shard over cores, run the SPMD bass kernel, return per-core results."""
    from concourse.bass_utils import run_bass_kernel_spmd

    nc = _get_program()
    in_maps = []
    for c in range(N_CORES):
        fshard = np.ascontiguousarray(feats_np[c * BS : (c + 1) * BS])
        lshard = labels_np[c * BS : (c + 1) * BS]
        ltile = np.ascontiguousarray(lshard.reshape(TILES, P).T.astype(np.float32))
        in_maps.append({"feats": fshard, "labels": ltile})
    kw = {}
    if trace:
        kw = {"trace": True}
    try:
        return run_bass_kernel_spmd(nc, in_maps, core_ids=list(range(N_CORES)), **kw)
    except Exception:
        # transient axon/terminal faults have been observed; retry once
        import time

        time.sleep(2.0)
        return run_bass_kernel_spmd(nc, in_maps, core_ids=list(range(N_CORES)), **kw)


def kernel(feats, centers, labels, _trace: bool = False, _return_res: bool = False):
    feats = np.asarray(feats, dtype=np.float32)
    centers = np.asarray(centers, dtype=np.float32)
    labels_i = np.asarray(labels).astype(np.int64)

    res = _run_device(feats, labels_i, trace=_trace)

    # host combine (the gather/unshard step): tiny [C, D] math
    sums_all = np.zeros((CPAD, D), dtype=np.float64)
    counts_all = np.zeros(CPAD, dtype=np.float64)
    S1 = 0.0
    for c in range(N_CORES):
        ev = res.results[c]["out_main"].astype(np.float64)
        # sums come out transposed: [feat_col, class] per (chunk, half)
        sums_all[0:512, 0:128] += ev[:, 0:512].T
        sums_all[512:1024, 0:128] += ev[:, 512:1024].T
        sums_all[0:512, 128:256] += ev[:, 1024:1536].T
        sums_all[512:1024, 128:256] += ev[:, 1536:2048].T
        S1 += np.trace(ev[:, 2048:2176]) + np.trace(ev[:, 2176:2304])
        counts_all += res.results[c]["out_cnt"][0].astype(np.float64)
    sums = sums_all[:C]
    counts = counts_all[:C]

    c64 = centers.astype(np.float64)
    A = float((sums * c64).sum())
    present = counts > 0
    X = float((np.square(sums).sum(axis=1)[present] / counts[present]).sum())
    W = float((counts * np.square(c64).sum(axis=1)).sum())
    loss = 0.5 / B * (S1 - 0.5 * A - 0.75 * X + 0.25 * W)
    out = np.float32(loss)
    if _return_res:
        return out, res
    return out
